# revision 7
# baseline (speedup 1.0000x reference)
"""Trainium2 Bass kernel for GNN message passing:

    h = segment_sum(x[src] * (edge_basis @ W.T + b), dst, num_segments=N)

Strategy (node-sharded, sort-based):
  - Host: sort edges by dst; core c owns the contiguous node range
    [c*N/8, (c+1)*N/8). Within a core, nodes are grouped into blocks of
    128; each block's (contiguous, because sorted) edge list is padded to
    tiles of 128 edges. Per core the host materializes:
      * ebT  [128, TT*128] bf16 : edge_basis of tile t, transposed (r on
        partitions, edges on free dim) -> streamed as matmul weights.
      * xs   [128, TT*64]  bf16 : x[src] gathered rows, edge on partition.
      * rel  [128, TT]     bf16 : dst - block_base per edge (0..127).
      * wt   [128, 64]     bf16 : W.T (loaded once, matmul rhs).
    Pad edges have ebT = 0, xs = 0, rel = 0 -> contribute exactly 0.
  - Device per tile (128 edges):
      PE:  filt[128e,64d](psum) = ebT_tile.T @ WT      (lhsT swapped per tile)
      ACT: filt -> bf16 SBUF (chunk-batched copy)      (+ bias if b != 0)
      DVE: m = xs * filt (bf16, chunk-batched)
      DVE: S[128e,128n] = one_hot(rel) via iota-compare (bf16, exact)
      PE:  psum_h[128n,64d] += S.T @ m                 (accumulate per block)
    Per block: ACT copies psum_h into a resident SBUF strip; one DMA at the
    end stores h [128, 49*64] f32; host de-interleaves to [N/8, 64].
  No collectives: each core owns its output rows exclusively.
"""

import math
from contextlib import ExitStack

import numpy as np
import ml_dtypes

import concourse.bass as bass
import concourse.bacc as bacc
import concourse.tile as tile
from concourse import mybir
from concourse.bass_utils import run_bass_kernel_spmd

BF16 = ml_dtypes.bfloat16

# Problem configuration (hardcoded per the task spec).
N_NODES = 50000
N_EDGES = 800000
D_IN = 64
D_RADIAL = 128
N_CORES = 8

LAST_BUILD = None  # (nc, in_maps) of the most recent build, for test harnesses

BLK = 128          # nodes per block (= one-hot width = psum partition dim)
CHUNK_MAX = 12     # max tiles per DVE/ACT batch (psum_filt <= 2 banks)


def _plan(dst_sorted, n_nodes, n_cores):
    """Compute per-(core, block) edge ranges and the shared tile schedule.

    Returns (T_list, e_start, e_end, npc, n_blocks):
      T_list[j]  = tiles allocated for local block j (same for all cores)
      e_start/e_end[c, j] = edge index range (into the sorted edge order)
    """
    npc = n_nodes // n_cores
    assert npc * n_cores == n_nodes
    n_blocks = math.ceil(npc / BLK)
    bounds = np.empty((n_cores, n_blocks + 1), np.int64)
    for c in range(n_cores):
        for j in range(n_blocks + 1):
            bounds[c, j] = c * npc + min(j * BLK, npc)
    e_bounds = np.searchsorted(dst_sorted, bounds.ravel()).reshape(bounds.shape)
    e_start = e_bounds[:, :-1]
    e_end = e_bounds[:, 1:]
    counts = e_end - e_start
    tiles_needed = np.maximum((counts + BLK - 1) // BLK, 1)
    T_list = tiles_needed.max(axis=0)  # max over cores -> shared schedule
    return T_list, e_start, e_end, npc, n_blocks


def _prepare_core(eb_bf, srcx, order, e_start, e_end, T_list,
                  core, npc, n_blocks, d_in, d_radial):
    """Build the per-core device input arrays (see module docstring)."""
    TT = int(T_list.sum())
    idx = np.full(TT * BLK, -1, np.int64)  # into sorted-edge order
    off = 0
    for j in range(n_blocks):
        s, e = e_start[core, j], e_end[core, j]
        idx[off * BLK: off * BLK + (e - s)] = order[s:e]
        off += int(T_list[j])
    pad = idx < 0
    idxc = np.where(pad, 0, idx)

    # ebT: [TT*128, d_radial] -> [d_radial, TT, 128] -> [d_radial, TT*128]
    ebg = eb_bf[idxc]
    ebg[pad] = 0
    ebT = np.ascontiguousarray(
        ebg.reshape(TT, BLK, d_radial).transpose(2, 0, 1).reshape(d_radial, TT * BLK))

    # xs: gathered source features [TT*128, d_in] -> [128, TT*64]
    xsg = srcx[idxc]                  # x[src] per original edge id
    xsg[pad] = 0
    xs = np.ascontiguousarray(
        xsg.reshape(TT, BLK, d_in).transpose(1, 0, 2).reshape(BLK, TT * d_in))
    return ebT, xs, idx, pad


def build_program(TT, T_list, n_blocks, d_in=D_IN, d_radial=D_RADIAL,
                  n_cores=N_CORES, has_bias=False):
    """Build + compile the SPMD Bass program (identical across cores)."""
    nc = bacc.Bacc("TRN2", target_bir_lowering=False, debug=False,
                   num_devices=n_cores)
    bf = mybir.dt.bfloat16
    f32 = mybir.dt.float32

    ebT_d = nc.dram_tensor("ebT", [d_radial, TT * BLK], bf, kind="ExternalInput")
    xs_d = nc.dram_tensor("xs", [BLK, TT * d_in], bf, kind="ExternalInput")
    rel_d = nc.dram_tensor("rel", [BLK, TT], f32, kind="ExternalInput")
    wt_d = nc.dram_tensor("wt", [d_radial, d_in], bf, kind="ExternalInput")
    if has_bias:
        bb_d = nc.dram_tensor("bb", [BLK, CHUNK_MAX * d_in], bf,
                              kind="ExternalInput")
    h_d = nc.dram_tensor("h", [BLK, n_blocks * d_in], f32, kind="ExternalOutput")

    T_cap = int(max(T_list))

    with TileContextCompat(nc) as tc, ExitStack() as ctx:
        const = ctx.enter_context(tc.tile_pool(name="const", bufs=1))
        ebp = ctx.enter_context(tc.tile_pool(name="ebp", bufs=3))
        xsp = ctx.enter_context(tc.tile_pool(name="xsp", bufs=3))
        fsb = ctx.enter_context(tc.tile_pool(name="fsb", bufs=3))
        msb = ctx.enter_context(tc.tile_pool(name="msb", bufs=3))
        ssb = ctx.enter_context(tc.tile_pool(name="ssb", bufs=8))
        pfil = ctx.enter_context(
            tc.tile_pool(name="pfil", bufs=2, space="PSUM"))
        ph = ctx.enter_context(tc.tile_pool(name="ph", bufs=2, space="PSUM"))

        wt_t = const.tile([d_radial, d_in], bf)
        nc.sync.dma_start(wt_t[:], wt_d.ap())
        rel_all = const.tile([BLK, TT], f32)
        nc.sync.dma_start(rel_all[:], rel_d.ap())
        if has_bias:
            bb_t = const.tile([BLK, CHUNK_MAX * d_in], bf)
            nc.sync.dma_start(bb_t[:], bb_d.ap())
        iota_i = const.tile([BLK, BLK], mybir.dt.int32)
        nc.gpsimd.iota(iota_i[:], pattern=[[1, BLK]], base=0,
                       channel_multiplier=0)
        iota_bf = const.tile([BLK, BLK], bf)
        nc.vector.tensor_copy(iota_bf[:], iota_i[:])
        h_all = const.tile([BLK, n_blocks * d_in], f32)

        off = 0
        for j in range(n_blocks):
            Tj = int(T_list[j])
            eb_t = ebp.tile([d_radial, T_cap * BLK], bf, tag="eb")
            nc.sync.dma_start(eb_t[:, :Tj * BLK],
                              ebT_d.ap()[:, off * BLK:(off + Tj) * BLK])
            xs_t = xsp.tile([BLK, T_cap * d_in], bf, tag="xs")
            nc.sync.dma_start(xs_t[:, :Tj * d_in],
                              xs_d.ap()[:, off * d_in:(off + Tj) * d_in])
            psum_h = ph.tile([BLK, d_in], f32)

            n_chunks = math.ceil(Tj / CHUNK_MAX)
            chunk = math.ceil(Tj / n_chunks)
            base = 0
            while base < Tj:
                cs = min(chunk, Tj - base)
                pf = pfil.tile([BLK, CHUNK_MAX * d_in], f32, tag="pf")
                for k in range(cs):
                    t = base + k
                    nc.tensor.matmul(pf[:, k * d_in:(k + 1) * d_in],
                                     eb_t[:, t * BLK:(t + 1) * BLK],
                                     wt_t[:], start=True, stop=True)
                f_sb = fsb.tile([BLK, CHUNK_MAX * d_in], bf, tag="f")
                nc.scalar.copy(f_sb[:, :cs * d_in], pf[:, :cs * d_in])
                if has_bias:
                    nc.vector.tensor_add(f_sb[:, :cs * d_in],
                                         f_sb[:, :cs * d_in],
                                         bb_t[:, :cs * d_in])
                m_sb = msb.tile([BLK, CHUNK_MAX * d_in], bf, tag="m")
                nc.vector.tensor_mul(
                    m_sb[:, :cs * d_in],
                    xs_t[:, base * d_in:(base + cs) * d_in],
                    f_sb[:, :cs * d_in])
                for k in range(cs):
                    t = base + k
                    s_t = ssb.tile([BLK, BLK], bf, tag="s")
                    nc.vector.tensor_scalar(
                        s_t[:], iota_bf[:],
                        rel_all[:, off + t:off + t + 1], None,
                        op0=mybir.AluOpType.is_equal)
                    nc.tensor.matmul(psum_h[:], s_t[:],
                                     m_sb[:, k * d_in:(k + 1) * d_in],
                                     start=(t == 0), stop=(t == Tj - 1))
                base += cs
            nc.scalar.copy(h_all[:, j * d_in:(j + 1) * d_in], psum_h[:])
            off += Tj
        nc.sync.dma_start(h_d.ap(), h_all[:])

    nc.compile()
    return nc


# TileContext wrapper: single place to tweak kwargs if needed.
def TileContextCompat(nc):
    return tile.TileContext(nc)


def _kernel_impl(x, edge_basis, src, dst, W, b,
                 n_nodes, d_in, d_radial, n_cores, run_fn=None):
    dst = np.asarray(dst)
    order = np.argsort(dst, kind="stable")
    dst_sorted = dst[order]
    T_list, e_start, e_end, npc, n_blocks = _plan(dst_sorted, n_nodes, n_cores)
    TT = int(T_list.sum())

    eb_bf = np.asarray(edge_basis).astype(BF16)
    srcx = np.asarray(x)[np.asarray(src)].astype(BF16)  # x gathered per edge

    has_bias = bool(np.any(np.asarray(b) != 0))

    in_maps = []
    for c in range(n_cores):
        ebT, xs, idx, pad = _prepare_core(
            eb_bf, srcx, order, e_start, e_end, T_list, c, npc, n_blocks,
            d_in, d_radial)
        # rel per slot: node index within the 128-node block; pads -> 0.
        rel_slot = np.zeros(TT * BLK, np.float32)
        valid = ~pad
        rel_slot[valid] = (dst[idx[valid]] - c * npc) % BLK
        rel_arr = np.ascontiguousarray(
            rel_slot.reshape(TT, BLK).T).astype(np.float32)
        m = {
            "ebT": ebT,
            "xs": xs,
            "rel": rel_arr,
            "wt": np.ascontiguousarray(np.asarray(W).T).astype(BF16),
        }
        if has_bias:
            m["bb"] = np.tile(np.asarray(b).astype(BF16), (BLK, CHUNK_MAX))
        in_maps.append(m)

    nc = build_program(TT, T_list, n_blocks, d_in, d_radial, n_cores,
                       has_bias)
    global LAST_BUILD
    LAST_BUILD = (nc, in_maps)
    if run_fn is None:
        res = run_bass_kernel_spmd(nc, in_maps, core_ids=list(range(n_cores)))
        results = res.results
    else:
        results = run_fn(nc, in_maps)

    h = np.empty((n_nodes, d_in), np.float32)
    for c in range(n_cores):
        hc = results[c]["h"].reshape(BLK, n_blocks, d_in)
        hc = hc.transpose(1, 0, 2).reshape(n_blocks * BLK, d_in)
        h[c * npc:(c + 1) * npc] = hc[:npc]
    return h


def kernel(x, edge_basis, src, dst, W, b):
    assert x.shape == (N_NODES, D_IN)
    assert edge_basis.shape == (N_EDGES, D_RADIAL)
    h = _kernel_impl(x, edge_basis, src, dst, W, b,
                     N_NODES, D_IN, D_RADIAL, N_CORES)
    return h.astype(x.dtype)
